# revision 1
# baseline (speedup 1.0000x reference)
"""Trainium2 Bass kernel for nn_AttentionHead (B=4, S=2048, H=D=1024, 8 cores).

Reference semantics (fp32):
    q = x @ Wq.T; k = x @ Wk.T; v = x @ Wv.T          (per batch b)
    kT = k.reshape(b, d, s)                            (raw reshape, NOT transpose)
    scores = q @ kT / sqrt(d)
    attn = softmax(scores, axis=0)                     (softmax over BATCH)
    attn_masked = where(tril(s, s), attn, 1e-9)
    out = attn_masked @ v

Sharding: the batch-softmax couples batches at identical (i, j), so all 4
batches of a given attention-map tile live on one core; cores shard the i
(query-row) axis.  Core c owns the 16-row blocks {8m + c : m = 0..15} (a
mod-8 staircase): at j-tile jt exactly the first jt of its 16 slots are fully
below the causal diagonal on EVERY core, so the SPMD program computes the
identical active suffix [16*jt : 256] everywhere, only the single boundary
slot needs a data mask (jj <= 16c + t, jt-independent), and the causal
compute is exact at 16-row granularity via ragged PSUM windows.  The same
rows form the core's k/v shard, so x is loaded once; k/v are projected
locally, rounded to fp16, and exchanged with per-batch AllGathers (k first -
scores need k before attn@v needs v).  Shards are stored in a permuted
[t, slot%2, slot//2, d] row order (free: the host stages x columns in that
order, so the k/v PSUM rows land in storage order and the shard write is a
plain copy; the q evacuation scatters its i columns back to slot order),
which makes both the kT = reshape(k) tiles and the v tiles plain 3-dim
strided reads of the gathered buffers, identical on every core.

Precision: single fp16 matmuls with fp32 PSUM accumulation for scores and
attn@v (operands are unit-scale; measured 3.6e-4 scale-relative error vs the
fp32 reference, ~50x inside the 2e-2 gate).  The projections run in fp32r
straight from the fp32 inputs (no operand rounding), and k/v/q round to fp16
only at PSUM evacuation.  The reference's post-mask 1e-9 fill contributes
less than 1.5e-7 absolute (1e-9 * column sums of v) and is dropped entirely;
masked attention entries are exactly 0 and fully-masked (i,j) tiles are
never computed or read.
"""

import numpy as np

B, S, H, D = 4, 2048, 1024, 1024
R = 8                  # cores
IB = 128               # j tile height
NJT = S // IB          # 16 j tiles of 128
ILOC = 256             # i rows per core (16 slots of 16 rows)
SL = ILOC              # k/v shard rows per core (same rows as q)

_CACHE = {}


def _subrows(c):
    """Global rows of core c in slot-ascending order (slot m = block 8m+c)."""
    return np.concatenate(
        [np.arange(16 * (8 * m + c), 16 * (8 * m + c) + 16) for m in range(16)]
    )


def _subrows_storage(c):
    """Global rows of core c in shard storage order l = 16t + 8*(slot%2) +
    slot//2 (the order the k/v shard is written to DRAM, chosen so the
    gathered kT and v reads are plain 3-dim strided access patterns)."""
    rows = np.empty(256, dtype=np.int64)
    for l in range(256):
        t, rem = divmod(l, 16)
        h2, msl = divmod(rem, 8)
        slot = 2 * msl + h2
        rows[l] = 16 * (8 * slot + c) + t
    return rows


def _build_program(sim=False):
    from contextlib import ExitStack

    import concourse.bacc as bacc
    import concourse.mybir as mybir
    from concourse import tile

    f32 = mybir.dt.float32
    f32r = mybir.dt.float32r
    f16 = mybir.dt.float16
    nc = bacc.Bacc("TRN2", target_bir_lowering=False, debug=False,
                   num_devices=(1 if sim else R))

    xt = nc.dram_tensor("xt", [B, H, ILOC], f32, kind="ExternalInput").ap()
    wqt = nc.dram_tensor("wqt", [H, D], f32, kind="ExternalInput").ap()
    wkt = nc.dram_tensor("wkt", [H, D], f32, kind="ExternalInput").ap()
    wvt = nc.dram_tensor("wvt", [H, D], f32, kind="ExternalInput").ap()
    m1 = nc.dram_tensor("m1", [IB, 16], f32, kind="ExternalInput").ap()
    out_loc = nc.dram_tensor("out_loc", [B, D, ILOC], f16,
                             kind="ExternalOutput").ap()

    with tile.TileContext(nc) as tc, ExitStack() as ctx:
        dram = ctx.enter_context(tc.tile_pool(name="dram", bufs=1, space="DRAM"))
        # shard rows stored as [t 16, slot%2, slot//2, d] (see kt/v reads)
        agi_k = dram.tile([B, SL, D], f16)
        agi_v = dram.tile([B, SL, D], f16)
        if sim:
            ag_k = [nc.dram_tensor(f"ag_k{b}", [R, SL, D], f16,
                                   kind="ExternalInput").ap() for b in range(B)]
            ag_v = [nc.dram_tensor(f"ag_v{b}", [R, SL, D], f16,
                                   kind="ExternalInput").ap() for b in range(B)]
        else:
            ag_k = [dram.tile([R, SL, D], f16, name=f"ag_k{b}")
                    for b in range(B)]
            ag_v = [dram.tile([R, SL, D], f16, name=f"ag_v{b}")
                    for b in range(B)]

        def all_gather(src_ap, dst_tile):
            nc.gpsimd.collective_compute(
                "AllGather", mybir.AluOpType.bypass,
                replica_groups=[list(range(R))],
                ins=[src_ap], outs=[dst_tile.opt()],
            )

        # ===================== projections (fp32r) ==========================
        qt_pool = ctx.enter_context(tc.tile_pool(name="qt", bufs=1))
        # kt tiles are consumed in the scores phase but their pool is opened
        # at top level so the kT reads stream during the projections.
        ktpool = ctx.enter_context(tc.tile_pool(name="ktpool", bufs=3))
        qt = []
        with tc.tile_pool(name="wpool", bufs=1) as wpool, \
             tc.tile_pool(name="xpool", bufs=1) as xpool:
            w_t = {}
            # wq shares wk's buffer (tag "wk"): its load starts right after
            # the k pass releases the weights and hides under the v pass
            for nm in ("wk", "wv"):
                w_t[nm] = wpool.tile([128, 8, D], f32r, tag=nm, name=nm)
            # x0 + first wk half lead: the first kv matmul chain only needs
            # these, so the PE starts ~10us earlier than with monolithic loads
            xts = []
            xb = xpool.tile([128, 8, ILOC], f32r, tag="x0", name="x0")
            xr0 = xt[0].rearrange("(t p) s -> p t s", p=128).bitcast(f32r)
            xts.append(xb)
            wkr = wkt.rearrange("(t p) d -> p t d", p=128).bitcast(f32r)
            # interleave x0 and wk d-half chunks by ht-half so the first kv
            # chain's ht 0-3 matmuls start after ~6us instead of ~12
            for hh in range(2):
                nc.sync.dma_start(
                    xb[:, 4 * hh:4 * (hh + 1), :], xr0[:, 4 * hh:4 * (hh + 1)]
                )
                nc.sync.dma_start(
                    w_t["wk"][:, 4 * hh:4 * (hh + 1), 0:512],
                    wkr[:, 4 * hh:4 * (hh + 1), 0:512],
                )
            for hh in range(2):
                nc.sync.dma_start(
                    w_t["wk"][:, 4 * hh:4 * (hh + 1), 512:1024],
                    wkr[:, 4 * hh:4 * (hh + 1), 512:1024],
                )
            for b in range(1, B):
                xb = xpool.tile([128, 8, ILOC], f32r, tag=f"x{b}", name=f"x{b}")
                nc.sync.dma_start(
                    xb[:], xt[b].rearrange("(t p) s -> p t s", p=128).bitcast(f32r)
                )
                xts.append(xb)
            nc.sync.dma_start(
                w_t["wv"][:], wvt.rearrange("(t p) d -> p t d", p=128).bitcast(f32r)
            )

            with tc.tile_pool(name="kvsb", bufs=6) as kvpool, \
                 tc.tile_pool(name="pskv", bufs=4, space="PSUM") as pskv:

                def proj_pass(wname, agi, agd):
                    wt = w_t[wname]
                    for b in range(B):
                        sb = kvpool.tile([128, 2, D], f16, tag="kv", name="kv")
                        for st in range(2):
                            for dblk in range(2):
                                ps = pskv.tile([128, 512], f32, tag="ps",
                                               name="ps")
                                for ht in range(8):
                                    nc.tensor.matmul(
                                        ps[:],
                                        xts[b][:, ht, 128 * st:128 * (st + 1)],
                                        wt[:, ht, dblk * 512:(dblk + 1) * 512],
                                        start=(ht == 0), stop=(ht == 7),
                                    )
                                nc.scalar.copy(
                                    sb[:, st, dblk * 512:(dblk + 1) * 512],
                                    ps[:],
                                )
                        nc.gpsimd.dma_start(
                            agi[b].rearrange("(st p) d -> p st d", p=128), sb[:]
                        )
                        if not sim:
                            all_gather(agi[b], agd[b])

                proj_pass("wk", agi_k, ag_k)

                w_t["wq"] = wpool.tile([128, 8, D], f32r, tag="wk", name="wq")
                nc.sync.dma_start(
                    w_t["wq"][:],
                    wqt.rearrange("(t p) d -> p t d", p=128).bitcast(f32r),
                )
                proj_pass("wv", agi_v, ag_v)

                # q projection, stored transposed as fp16 [d, i] (slot order)
                wt = w_t["wq"]
                for b in range(B):
                    qb = qt_pool.tile([128, 8, ILOC], f16, tag=f"q{b}",
                                      name=f"q{b}")
                    qt.append(qb)
                    for mt in range(8):
                        ps = pskv.tile([128, 512], f32, tag="ps", name="ps")
                        for ht in range(8):
                            nc.tensor.matmul(
                                ps[:, 0:ILOC],
                                wt[:, ht, mt * 128:(mt + 1) * 128],
                                xts[b][:, ht, :],
                                start=(ht == 0), stop=(ht == 7),
                            )
                        # x columns arrive in shard storage order
                        # l = 16t + 8*(slot%2) + slot//2; scatter the i
                        # columns back to slot order 16*slot + t here so the
                        # scores/attn causal suffix stays contiguous
                        nc.scalar.copy(
                            qb[:, mt, :].rearrange(
                                "p (msl h2 t) -> p t h2 msl",
                                msl=8, h2=2, t=16,
                            ),
                            ps[:, 0:ILOC],
                        )

        # ============== scores (transposed) + exp + batch softmax ===========
        # ah[b][jt] holds attn.T tile [j, i] in fp16; only the active causal
        # suffix [16*jt:] is ever written or read.
        with tc.tile_pool(name="ahpool", bufs=B * NJT) as ahpool, \
             tc.tile_pool(name="epool", bufs=30) as epool, \
             tc.tile_pool(name="denp", bufs=9) as denp, \
             tc.tile_pool(name="rmp", bufs=3) as rmp, \
             tc.tile_pool(name="mpool", bufs=1) as mpool, \
             tc.tile_pool(name="vpool", bufs=2) as vpool, \
             tc.tile_pool(name="opool", bufs=2) as opool:
            ah = [[None] * NJT for _ in range(B)]
            m1_sb = mpool.tile([IB, 16], f32, tag="m1")
            nc.sync.dma_start(m1_sb[:], m1)
            e_grp = {}
            den_grp = {}
            vts = {}

            def v_load(b, eng):
                # v rows of tile jt: core w, slot jt, t = row%16; the
                # (jt//2, d) block is one contiguous run per (w, t).  Tiles
                # are split at jt//2 = 4 so attn@v can start on a quarter of
                # the batch's v bytes and the rest streams under compute.
                tiles = {}
                for nh in range(2):
                    for par in range(2):  # jt parity: slot%2 = jt%2
                        vt = vpool.tile([128, 4, D], f16, tag=f"v{par}{nh}",
                                        name=f"v{par}{nh}")
                        eng.dma_start(
                            vt[:],
                            ag_v[b].rearrange(
                                "u (t hh mt) d -> hh u t mt d",
                                t=16, hh=2, mt=8,
                            )[par, :, :, 4 * nh:4 * (nh + 1)],
                        )
                        tiles[(par, nh)] = vt
                vts[b] = tiles

            with tc.tile_pool(name="pss", bufs=4, space="PSUM") as pss:
              for jh in range(2):         # halves of 8 j-tiles (1024 j rows)
                for b in range(B):
                    # kT tile: kt[dl, mt, jcol] = kT[128*mt + dl,
                    # 1024*jh + jcol]; partition dl = (u, p) with k-row =
                    # 256*mt + 16*u + 2*p + jh, i.e. core u%8, slot
                    # 2*mt + u//8, t = 2*p + jh.  In shard storage order the
                    # (mt, jcol) block is one contiguous 8192 run; the u//8
                    # halves differ by a +8192 offset.
                    kt = ktpool.tile([128, 8, D], f16, tag="kt", name="kt")
                    src = ag_k[b].rearrange(
                        "u (t hh mt) d -> t hh u mt d", t=16, hh=2, mt=8,
                    )
                    for h2 in range(2):
                        nc.sync.dma_start(
                            kt[64 * h2:64 * (h2 + 1)],
                            src[jh::2, h2].rearrange("t u mt d -> u t mt d"),
                        )
                    for jq in range(8):
                        jt = 8 * jh + jq
                        io = 16 * jt
                        w = ILOC - io
                        ps = pss.tile([128, w], f32, tag="ps", name="ps")
                        for mt in range(8):
                            nc.tensor.matmul(
                                ps[:],
                                kt[:, mt, jq * 128:(jq + 1) * 128],
                                qt[b][:, mt, io:io + w],
                                start=(mt == 0), stop=(mt == 7),
                            )
                        e = epool.tile([IB, ILOC], f16, tag="e", name="e")
                        nc.scalar.activation(
                            e[:, io:io + w], ps[:],
                            mybir.ActivationFunctionType.Exp,
                            scale=float(1.0 / np.sqrt(D)),
                        )
                        e_grp[(b, jt)] = e
                        # denominator folds in as each batch's e arrives, so
                        # only one add sits on the b==3 critical chain
                        if b == 1:
                            den = denp.tile([IB, ILOC], f32, tag="den",
                                            name="den")
                            den_grp[jt] = den
                            nc.vector.tensor_add(
                                den[:, io:io + w],
                                e_grp[(0, jt)][:, io:io + w],
                                e[:, io:io + w],
                            )
                        elif b >= 2:
                            den = den_grp[jt]
                            nc.vector.tensor_add(
                                den[:, io:io + w], den[:, io:io + w],
                                e[:, io:io + w],
                            )
                        if b < B - 1:
                            continue
                        # ---- softmax over batch + causal mask + fp16 -------
                        rm = rmp.tile([IB, ILOC], f32, tag="rm", name="rm")
                        nc.vector.reciprocal(rm[:, io:io + w],
                                             den[:, io:io + w])
                        # boundary slot: zero attn where j > i
                        nc.vector.tensor_mul(
                            rm[:, io:io + 16], rm[:, io:io + 16], m1_sb[:]
                        )
                        for bb in range(B):
                            a = ahpool.tile([IB, ILOC], f16, tag="ah",
                                            name="ah")
                            nc.vector.tensor_mul(
                                a[:, io:io + w],
                                e_grp[(bb, jt)][:, io:io + w],
                                rm[:, io:io + w],
                            )
                            ah[bb][jt] = a

            # ===================== attn.T @ v (out is [d, i]) ===============
            # Ragged causal accumulation: psum column block [16t, 16t+16)
            # gets its last contribution at jt = t; start covers the full
            # width at jt = 0 (every column is causally active there).
            # v loads for b0/b1 ride the SP queue BEHIND the kt stream, so
            # their transfers can't cut ahead of the jh=1 kT tiles in the
            # DMA FIFO; b2/b3 are emitted inside the attn@v loop on the Act
            # queue exactly where their buffer-reuse waits resolve.  (Real
            # hardware: nothing later on SP depends on the v gathers, and
            # the agi_v shard writes feeding them are on the Pool queue, so
            # no deadlock.)
            v_load(0, nc.sync)
            v_load(1, nc.sync)
            with tc.tile_pool(name="pso", bufs=2, space="PSUM") as pso:
              for b in range(B):
                pss_o = [pso.tile([128, 2, ILOC], f32, tag=f"o{g}",
                                  name=f"o{g}") for g in range(4)]
                # a PSUM bank holds a single accumulation group, so the two
                # halves of each bank-pair accumulate in separate jt sweeps
                for h in range(2):
                    for jt in range(NJT):
                        io = 16 * jt
                        w = ILOC - io
                        vt = vts[b][(jt % 2, jt // 8)][:, (jt // 2) % 4, :]
                        a = ah[b][jt]
                        for g in range(4):
                            dc = 2 * g + h
                            nc.tensor.matmul(
                                pss_o[g][:, h, io:io + w],
                                vt[:, dc * 128:(dc + 1) * 128],
                                a[:, io:io + w],
                                start=(jt == 0), stop=(jt == NJT - 1),
                                skip_group_check=True,
                            )
                if b + 2 < B:
                    v_load(b + 2, nc.scalar)
                osb = opool.tile([128, 8, ILOC], f16, tag="osb", name="osb")
                for dc in range(8):
                    # evacuation split across Act/DVE so the next batch's
                    # accumulation isn't gated on a single engine
                    if dc < 4:
                        nc.scalar.copy(osb[:, dc, :],
                                       pss_o[dc // 2][:, dc % 2, :])
                    else:
                        nc.vector.tensor_copy(osb[:, dc, :],
                                              pss_o[dc // 2][:, dc % 2, :])
                if b < B - 1:
                    nc.gpsimd.dma_start(
                        out_loc[b].rearrange("(dc p) i -> p dc i", p=128),
                        osb[:],
                    )
                else:
                    # last batch: the write is the kernel tail, so ship the
                    # Act-evacuated half and the DVE-evacuated half as two
                    # engine-parallel DMAs, each leaving as its evacs finish
                    for oh in range(2):
                        eng = nc.scalar if oh == 0 else nc.gpsimd
                        eng.dma_start(
                            out_loc[b][512 * oh:512 * (oh + 1)].rearrange(
                                "(dc p) i -> p dc i", p=128
                            ),
                            osb[:, 4 * oh:4 * (oh + 1), :],
                        )

    nc.compile()
    return nc


def _host_inputs(x, Wq, Wk, Wv):
    x = np.ascontiguousarray(x, dtype=np.float32)
    wqt = np.ascontiguousarray(Wq.T, dtype=np.float32)
    wkt = np.ascontiguousarray(Wk.T, dtype=np.float32)
    wvt = np.ascontiguousarray(Wv.T, dtype=np.float32)

    in_maps = []
    jj = np.arange(IB)[:, None]
    t = np.arange(16)[None, :]
    for c in range(R):
        rows = _subrows_storage(c)
        xtc = np.ascontiguousarray(x[:, rows, :].transpose(0, 2, 1))
        # boundary slot jt (global rows 16*(8*jt+c) + t, j = 128*jt + jj):
        # keep j <= i  <=>  jj <= 16*c + t   (jt-independent)
        m1 = (jj <= 16 * c + t).astype(np.float32)
        in_maps.append({
            "xt": xtc, "wqt": wqt, "wkt": wkt, "wvt": wvt,
            "m1": np.ascontiguousarray(m1),
        })
    return in_maps


def kernel(x, Wq, Wk, Wv):
    from concourse.bass_utils import run_bass_kernel_spmd

    if "nc" not in _CACHE:
        _CACHE["nc"] = _build_program()
    nc = _CACHE["nc"]

    in_maps = _host_inputs(x, Wq, Wk, Wv)
    res = None
    for attempt in range(3):
        try:
            res = run_bass_kernel_spmd(nc, in_maps, list(range(R)))
            break
        except Exception:
            # transient NRT_EXEC_UNIT_UNRECOVERABLE wedges recover on retry
            if attempt == 2:
                raise
            import time
            time.sleep(15)

    out = np.empty((B, S, D), dtype=np.float32)
    for c in range(R):
        out[:, _subrows(c), :] = res.results[c]["out_loc"].transpose(0, 2, 1)
    return out


if __name__ == "__main__":
    rng = np.random.default_rng(0)
    x = rng.standard_normal((B, S, H), dtype=np.float32)
    Wq = rng.standard_normal((D, H), dtype=np.float32) / np.sqrt(H)
    Wk = rng.standard_normal((D, H), dtype=np.float32) / np.sqrt(H)
    Wv = rng.standard_normal((D, H), dtype=np.float32) / np.sqrt(H)
    o = kernel(x, Wq, Wk, Wv)
    print("kernel output", o.shape, o.dtype, float(np.abs(o).max()))



# revision 27
# speedup vs baseline: 1.0713x; 1.0713x over previous
"""Trainium2 Bass kernel for nn_AttentionHead (B=4, S=2048, H=D=1024, 8 cores).

Reference semantics (fp32):
    q = x @ Wq.T; k = x @ Wk.T; v = x @ Wv.T          (per batch b)
    kT = k.reshape(b, d, s)                            (raw reshape, NOT transpose)
    scores = q @ kT / sqrt(d)
    attn = softmax(scores, axis=0)                     (softmax over BATCH)
    attn_masked = where(tril(s, s), attn, 1e-9)
    out = attn_masked @ v

Sharding: the batch-softmax couples batches at identical (i, j), so all 4
batches of a given attention-map tile live on one core; cores shard the i
(query-row) axis.  Core c owns the 16-row blocks {8m + c : m = 0..15} (a
mod-8 staircase): at j-tile jt exactly the first jt of its 16 slots are fully
below the causal diagonal on EVERY core, so the SPMD program computes the
identical active suffix [16*jt : 256] everywhere, only the single boundary
slot needs a data mask (jj <= 16c + t, jt-independent), and the causal
compute is exact at 16-row granularity via ragged PSUM windows.  The same
rows form the core's k/v shard, so x is loaded once; k/v are projected
locally, rounded to fp16, and exchanged with per-batch AllGathers (k first -
scores need k before attn@v needs v).  Shards are stored in a permuted
[t, slot%2, slot//2, d] row order so the kT = reshape(k) tiles and the v
tiles are plain strided reads of the gathered buffers, identical on every
core (the host stages x columns in that order; the q evacuation scatters its
i columns back to slot order).

Phase schedule (PE order), chosen so the 48 MB/core of HBM traffic streams
under the ~140 us of fp16 matmul instead of piling into the last third:
    1. k-projection (4b)                    | x + Wk/Wq fp16 loads
    2. q-projection (4b)                    | Wv load, kT prefetch
    3. per b: scores jh=0 ; v-projection    | kT stream, v-piece prefetch ring
    4. scores jh=1 (4b)                     | v-piece ring fills
    5. attn.T@v, g-major per (b, dc-pair)   | v tail + per-group out DMA
Inputs are staged fp16 on the host (halves input DMA); e/den softmax tiles
are ragged fp16 with in-place normalization (2x DVE, ~4 MB SBUF saved) which
funds a 9-slot x 1 MB gathered-v prefetch ring; attn@v finishes each dc-pair
group separately so output DMA trickles out instead of forming a tail.

Precision: fp16 operands with fp32 PSUM accumulation everywhere (x/W rounded
on the host; k/v/q rounded at PSUM evacuation).  The fp16 denominator adds
~1e-3 relative error; measured ~1.5e-3 total vs the fp32 reference, ~13x
inside the 2e-2 gate.  The reference's post-mask 1e-9 fill contributes
< 1.5e-7 absolute and is dropped; masked entries are exactly 0 and fully
masked (i,j) tiles are never computed.
"""

import numpy as np

B, S, H, D = 4, 2048, 1024, 1024
R = 8                  # cores
IB = 128               # j tile height
NJT = S // IB          # 16 j tiles of 128
ILOC = 256             # i rows per core (16 slots of 16 rows)
SL = ILOC              # k/v shard rows per core (same rows as q)
NVG = 8                # gathered-v prefetch ring slots (1 MB each)

_CACHE = {}


def _subrows(c):
    """Global rows of core c in slot-ascending order (slot m = block 8m+c)."""
    return np.concatenate(
        [np.arange(16 * (8 * m + c), 16 * (8 * m + c) + 16) for m in range(16)]
    )


def _subrows_storage(c):
    """Global rows of core c in shard storage order l = 16t + 8*(slot%2) +
    slot//2 (the order the k/v shard is written to DRAM, chosen so the
    gathered kT and v reads are plain strided access patterns)."""
    rows = np.empty(256, dtype=np.int64)
    for l in range(256):
        t, rem = divmod(l, 16)
        h2, msl = divmod(rem, 8)
        slot = 2 * msl + h2
        rows[l] = 16 * (8 * slot + c) + t
    return rows


def _build_program(sim=False):
    from contextlib import ExitStack

    import concourse.bacc as bacc
    import concourse.mybir as mybir
    from concourse import tile

    f32 = mybir.dt.float32
    f16 = mybir.dt.float16
    nc = bacc.Bacc("TRN2", target_bir_lowering=False, debug=False,
                   num_devices=(1 if sim else R))

    xt = nc.dram_tensor("xt", [B, H, ILOC], f16, kind="ExternalInput").ap()
    wqt = nc.dram_tensor("wqt", [H, D], f16, kind="ExternalInput").ap()
    wkt = nc.dram_tensor("wkt", [H, D], f16, kind="ExternalInput").ap()
    wvt = nc.dram_tensor("wvt", [H, D], f16, kind="ExternalInput").ap()
    m1 = nc.dram_tensor("m1", [IB, 16], f16, kind="ExternalInput").ap()
    out_loc = nc.dram_tensor("out_loc", [B, D, ILOC], f16,
                             kind="ExternalOutput").ap()

    with tile.TileContext(nc) as tc, ExitStack() as ctx:
        dram = ctx.enter_context(tc.tile_pool(name="dram", bufs=1, space="DRAM"))
        # shard rows stored as [t 16, slot%2, slot//2, d] (see kt/vg reads)
        agi_k = dram.tile([B, SL, D], f16)
        agi_v = dram.tile([B, SL, D], f16)
        if sim:
            ag_k = [nc.dram_tensor(f"ag_k{b}", [R, SL, D], f16,
                                   kind="ExternalInput").ap() for b in range(B)]
            ag_v = [nc.dram_tensor(f"ag_v{b}", [R, SL, D], f16,
                                   kind="ExternalInput").ap() for b in range(B)]
        else:
            ag_k = [dram.tile([R, SL, D], f16, name=f"ag_k{b}")
                    for b in range(B)]
            ag_v = [dram.tile([R, SL, D], f16, name=f"ag_v{b}")
                    for b in range(B)]

        def all_gather(src_ap, dst_tile):
            nc.gpsimd.collective_compute(
                "AllGather", mybir.AluOpType.bypass,
                replica_groups=[list(range(R))],
                ins=[src_ap], outs=[dst_tile.opt()],
            )

        # ---- persistent pools (whole-kernel lifetime) ----------------------
        ktpool = ctx.enter_context(tc.tile_pool(name="ktpool", bufs=3))
        qt_pool = ctx.enter_context(tc.tile_pool(name="qt", bufs=1))
        epool = ctx.enter_context(tc.tile_pool(name="epool", bufs=B))
        denp = ctx.enter_context(tc.tile_pool(name="denp", bufs=1))
        mpool = ctx.enter_context(tc.tile_pool(name="mpool", bufs=1))
        opool = ctx.enter_context(tc.tile_pool(name="opool", bufs=4))
        pss = ctx.enter_context(tc.tile_pool(name="pss", bufs=3, space="PSUM"))

        m1_sb = mpool.tile([IB, 16], f16, tag="m1")
        qt = []
        e_grp = {}
        den_grp = {}
        vgs = {}
        pools = {}
        vg_queue = []  # (b, g) pieces not yet issued, in consumption order

        def vg_issue(n, eng):
            """Issue the next n gathered-v piece loads.  Piece (b, g) is
            [j-row-in-tile 128, jt 16, dc-pair cols 256].  The first NVG
            pieces are ring-wait-free and ride the DVE queue (never behind
            throttled kT tiles in the SP FIFO); later pieces carry ring-reuse
            waits that would head-block DVE's softmax ops (which attn@v --
            the ring consumer -- depends on), so they go to SP, which is
            idle once the last kT tile is issued."""
            for _ in range(n):
                if not vg_queue:
                    return
                b, g = vg_queue.pop(0)
                vt = pools["vg"].tile([128, 8, 2, 256], f16, tag="vg",
                                      name=f"vg{b}{g}")
                src = ag_v[b].rearrange(
                    "u (t par msl) d -> (u t) msl par d",
                    t=16, par=2, msl=8,
                )[:, :, :, 256 * g:256 * (g + 1)]
                for par in range(2):  # DMA APs allow at most 3 free dims
                    eng.dma_start(vt[:, :, par, :], src[:, :, par, :])
                vgs[(b, g)] = vt

        def scores_block(jh, b):
            # kT tile: kt[dl, mt, jcol] = kT[128*mt + dl, 1024*jh + jcol];
            # partition dl = (u, p) with k-row = 256*mt + 16*u + 2*p + jh.
            # In shard storage order the (mt, jcol) block is one contiguous
            # 8192 run; the u//8 halves differ by a +8192 offset.
            kt = ktpool.tile([128, 8, D], f16, tag="kt", name="kt")
            src = ag_k[b].rearrange(
                "u (t hh mt) d -> t hh u mt d", t=16, hh=2, mt=8,
            )
            for h2 in range(2):
                nc.sync.dma_start(
                    kt[64 * h2:64 * (h2 + 1)],
                    src[jh::2, h2].rearrange("t u mt d -> u t mt d"),
                )
            for jq in range(8):
                jt = 8 * jh + jq
                io = 16 * jt
                w = ILOC - io
                ps = pss.tile([128, w], f32, tag="ps", name="ps")
                for mt in range(8):
                    nc.tensor.matmul(
                        ps[:],
                        kt[:, mt, jq * 128:(jq + 1) * 128],
                        qt[b][:, mt, io:io + w],
                        start=(mt == 0), stop=(mt == 7),
                    )
                # e tile is ragged: local column c is global i-column io + c
                e = epool.tile([IB, w], f16, tag=f"e{jt}", name=f"e{jt}")
                nc.scalar.activation(
                    e[:], ps[:],
                    mybir.ActivationFunctionType.Exp,
                    scale=float(1.0 / np.sqrt(D)),
                )
                e_grp[(b, jt)] = e
                # denominator folds in as each batch's e arrives, so only
                # one add sits on the b==3 critical chain (all fp16: 2x DVE)
                if b == 1:
                    den = denp.tile([IB, w], f16, tag=f"den{jt}",
                                    name=f"den{jt}")
                    den_grp[jt] = den
                    nc.vector.tensor_add(den[:], e_grp[(0, jt)][:], e[:])
                elif b >= 2:
                    den = den_grp[jt]
                    nc.vector.tensor_add(den[:], den[:], e[:])
                if b < B - 1:
                    continue
                # ---- softmax over batch + causal mask, all in place --------
                # fp16 denominator: ~1e-3 relative on attn, 13x inside gate
                with nc.allow_low_precision(reason="fp16 softmax denom"):
                    nc.vector.reciprocal(den[:], den[:])
                # boundary slot (local cols 0:16): zero attn where j > i
                nc.vector.tensor_mul(den[:, 0:16], den[:, 0:16], m1_sb[:])
                for bb in range(B):
                    ebb = e_grp[(bb, jt)]
                    nc.vector.tensor_mul(ebb[:], ebb[:], den[:])

        # ==================== phases 1-3: projections =======================
        with tc.tile_pool(name="wpool_b", bufs=1) as wpool_b, \
             tc.tile_pool(name="xpool", bufs=1) as xpool, \
             tc.tile_pool(name="kvsb", bufs=4) as kvpool, \
             tc.tile_pool(name="pskv", bufs=3, space="PSUM") as pskv:

            xts = []
            for b in range(B):
                xts.append(xpool.tile([128, 8, ILOC], f16, tag=f"x{b}",
                                      name=f"x{b}"))
            xr0 = xt[0].rearrange("(t p) s -> p t s", p=128)
            wkr = wkt.rearrange("(t p) d -> p t d", p=128)

            def proj_kv(wt, b, agi):
                sb = kvpool.tile([128, 2, D], f16, tag="kv", name="kv")
                for st in range(2):
                    for dblk in range(2):
                        ps = pskv.tile([128, 512], f32, tag="ps", name="ps")
                        for ht in range(8):
                            nc.tensor.matmul(
                                ps[:],
                                xts[b][:, ht, 128 * st:128 * (st + 1)],
                                wt[:, ht, dblk * 512:(dblk + 1) * 512],
                                start=(ht == 0), stop=(ht == 7),
                            )
                        nc.scalar.copy(
                            sb[:, st, dblk * 512:(dblk + 1) * 512], ps[:],
                        )
                nc.gpsimd.dma_start(
                    agi[b].rearrange("(st p) d -> p st d", p=128), sb[:]
                )

            with tc.tile_pool(name="wpool_a", bufs=1) as wpool_a:
                wk = wpool_a.tile([128, 8, D], f16, tag="w1", name="wk")
                wq = wpool_a.tile([128, 8, D], f16, tag="w2", name="wq")
                # x0 + wk d-half chunks interleaved by ht-half: the first kv
                # matmul chain starts after ~3 us instead of ~12
                for hh in range(2):
                    nc.sync.dma_start(
                        xts[0][:, 4 * hh:4 * (hh + 1), :],
                        xr0[:, 4 * hh:4 * (hh + 1)],
                    )
                    nc.sync.dma_start(
                        wk[:, 4 * hh:4 * (hh + 1), 0:512],
                        wkr[:, 4 * hh:4 * (hh + 1), 0:512],
                    )
                for hh in range(2):
                    nc.sync.dma_start(
                        wk[:, 4 * hh:4 * (hh + 1), 512:1024],
                        wkr[:, 4 * hh:4 * (hh + 1), 512:1024],
                    )
                for b in range(1, B):
                    nc.sync.dma_start(
                        xts[b][:], xt[b].rearrange("(t p) s -> p t s", p=128)
                    )
                for dh in range(2):  # halves so small DMAs behind never
                    nc.sync.dma_start(  # sit behind one 5.8 us transfer
                        wq[:, :, 512 * dh:512 * (dh + 1)],
                        wqt.rearrange("(t p) d -> p t d", p=128)[
                            :, :, 512 * dh:512 * (dh + 1)],
                    )
                nc.sync.dma_start(m1_sb[:], m1)

                # -------- phase 1: k-projection + per-batch AllGather -------
                for b in range(B):
                    proj_kv(wk, b, agi_k)
                    if not sim:
                        all_gather(agi_k[b], ag_k[b])

                # wv lives in the long-lived pool (phase-3 v-pass reads it
                # after wpool_a closes); its load streams during the q pass
                wv = wpool_b.tile([128, 8, D], f16, tag="wv", name="wv")
                nc.sync.dma_start(
                    wv[:], wvt.rearrange("(t p) d -> p t d", p=128)
                )

                # -------- phase 2: q-projection (stored transposed) ---------
                for b in range(B):
                    qb = qt_pool.tile([128, 8, ILOC], f16, tag=f"q{b}",
                                      name=f"q{b}")
                    qt.append(qb)
                    # two mt accumulation windows share one PSUM bank, so the
                    # 3-deep evac ring turns over per 1.7 us (not 0.85) and
                    # never gates the PE
                    for mtp in range(4):
                        ps = pskv.tile([128, 512], f32, tag="ps", name="ps")
                        for half in range(2):
                            mt = 2 * mtp + half
                            for ht in range(8):
                                nc.tensor.matmul(
                                    ps[:, 256 * half:256 * half + ILOC],
                                    wq[:, ht, mt * 128:(mt + 1) * 128],
                                    xts[b][:, ht, :],
                                    start=(ht == 0), stop=(ht == 7),
                                    skip_group_check=True,
                                )
                        # x columns arrive in shard storage order
                        # l = 16t + 8*(slot%2) + slot//2; scatter the i
                        # columns back to slot order 16*slot + t here so the
                        # scores/attn causal suffix stays contiguous
                        for half in range(2):
                            mt = 2 * mtp + half
                            nc.scalar.copy(
                                qb[:, mt, :].rearrange(
                                    "p (msl h2 t) -> p t h2 msl",
                                    msl=8, h2=2, t=16,
                                ),
                                ps[:, 256 * half:256 * half + ILOC],
                            )

            # wpool_a closed: wq's slot freed funds the v prefetch ring.
            # Pool closes are strictly LIFO, so phases 4-5 stay inside this
            # block (wpool_b/xpool/kvpool idle 44 KB/partition there, which
            # still fits: ~203 of 208 KB at peak).
            with tc.tile_pool(name="vgpool", bufs=NVG) as vgpool:
              pools["vg"] = vgpool
              vg_queue.extend([(b, g) for b in range(B) for g in range(4)])

              # ---- phase 3: per batch, scores jh=0 then v-projection -------
              for b in range(B):
                scores_block(0, b)
                proj_kv(wv, b, agi_v)
                if not sim:
                    all_gather(agi_v[b], ag_v[b])
                # pieces of batch b issue after its gather; exactly NVG=9
                # ring-wait-free pieces interleave with the kT stream here
                vg_issue(4 if b < 2 else (1 if b == 2 else 0), nc.sync)

              # ================== phase 4: scores jh=1 ======================
              for b in range(B):
                scores_block(1, b)
              # pieces 9-15 carry ring-reuse waits (freed by attn@v consuming
              # earlier pieces); issuing them after the LAST kT tile keeps
              # those waits from head-blocking the kT stream scores needs
              vg_issue(7, nc.sync)

              # ================== phase 5: attn.T @ v =======================
              # g-major: each dc-pair group accumulates its ragged causal jt
              # sweep (psum column block [16t, 16t+16) gets its last
              # contribution at jt = t), then evacuates split across Act/DVE
              # and ships its own 128 KB of output immediately -- no
              # end-of-kernel DMA tail.
              with tc.tile_pool(name="pso", bufs=2, space="PSUM") as pso:
                for b in range(B):
                    for g in range(4):
                        vt = vgs[(b, g)]
                        ps = pso.tile([128, 2, ILOC], f32, tag="o", name="o")
                        for h in range(2):
                            for jt in range(NJT):
                                io = 16 * jt
                                w = ILOC - io
                                a = e_grp[(b, jt)]
                                nc.tensor.matmul(
                                    ps[:, h, io:io + w],
                                    vt[:, jt // 2, jt % 2,
                                       128 * h:128 * (h + 1)],
                                    a[:],
                                    start=(jt == 0), stop=(jt == NJT - 1),
                                    skip_group_check=True,
                                )
                        osb = opool.tile([128, 2, ILOC], f16, tag="osb",
                                         name="osb")
                        nc.scalar.copy(osb[:, 0, :], ps[:, 0, :])
                        nc.vector.tensor_copy(osb[:, 1, :], ps[:, 1, :])
                        nc.gpsimd.dma_start(
                            out_loc[b][256 * g:256 * (g + 1)].rearrange(
                                "(dc p) i -> p dc i", p=128
                            ),
                            osb[:],
                        )

    nc.compile()
    return nc


def _host_inputs(x, Wq, Wk, Wv):
    x = np.asarray(x, dtype=np.float32).astype(np.float16)
    wqt = np.ascontiguousarray(np.asarray(Wq, dtype=np.float32).T.astype(np.float16))
    wkt = np.ascontiguousarray(np.asarray(Wk, dtype=np.float32).T.astype(np.float16))
    wvt = np.ascontiguousarray(np.asarray(Wv, dtype=np.float32).T.astype(np.float16))

    in_maps = []
    jj = np.arange(IB)[:, None]
    t = np.arange(16)[None, :]
    for c in range(R):
        rows = _subrows_storage(c)
        xtc = np.ascontiguousarray(x[:, rows, :].transpose(0, 2, 1))
        # boundary slot jt (global rows 16*(8*jt+c) + t, j = 128*jt + jj):
        # keep j <= i  <=>  jj <= 16*c + t   (jt-independent)
        m1 = (jj <= 16 * c + t).astype(np.float16)
        in_maps.append({
            "xt": xtc, "wqt": wqt, "wkt": wkt, "wvt": wvt,
            "m1": np.ascontiguousarray(m1),
        })
    return in_maps


def kernel(x, Wq, Wk, Wv):
    from concourse.bass_utils import run_bass_kernel_spmd

    if "nc" not in _CACHE:
        _CACHE["nc"] = _build_program()
    nc = _CACHE["nc"]

    in_maps = _host_inputs(x, Wq, Wk, Wv)
    res = None
    for attempt in range(3):
        try:
            res = run_bass_kernel_spmd(nc, in_maps, list(range(R)))
            break
        except Exception:
            # transient NRT_EXEC_UNIT_UNRECOVERABLE wedges recover on retry
            if attempt == 2:
                raise
            import time
            time.sleep(15)

    out = np.empty((B, S, D), dtype=np.float32)
    for c in range(R):
        out[:, _subrows(c), :] = res.results[c]["out_loc"].transpose(0, 2, 1)
    return out


if __name__ == "__main__":
    rng = np.random.default_rng(0)
    x = rng.standard_normal((B, S, H), dtype=np.float32)
    Wq = rng.standard_normal((D, H), dtype=np.float32) / np.sqrt(H)
    Wk = rng.standard_normal((D, H), dtype=np.float32) / np.sqrt(H)
    Wv = rng.standard_normal((D, H), dtype=np.float32) / np.sqrt(H)
    o = kernel(x, Wq, Wk, Wv)
    print("kernel output", o.shape, o.dtype, float(np.abs(o).max()))
